# revision 18
# baseline (speedup 1.0000x reference)
"""Trainium2 Bass kernel for nn_EncoderLayer_11132555231236.

Computation (reference.py): two self-attentions over value[:, :, 0/1]
(Q/K/V all derived from the value tensor via shared per-head 64x64
projections), summed, +query residual, LN, FFN(1024->4096->1024), +res, LN.

Sharding: 8 cores = (batch b in {0,1}) x (query-row slice qs in {0..3},
512 rows each of L=2048). No collectives: each core computes K/V for the
full sequence of its batch (cheap per-head 64x64 projections) and its own
512 query rows end-to-end through the FFN.

Dataflow per core (all matmuls in float32r = TF32-rate, fp32 accumulate):
  - x_aug = [x | ones] DMA'd per head-block (ones column yields softmax
    sums for free via the att.T @ x_aug matmul).
  - PE-transpose x blocks -> xT per head-pair; kT = blockdiag(Wk,Wk).T@xT,
    qT likewise (2 heads per 128-contraction matmul).
  - energyT[kk, q] = kT.T @ qT per 128-kk tile; exp on ScalarE (scale
    1/sqrt(E)); unnormalized out1T[c,q] (+sums row) = x_aug.T @ attT
    accumulated over kk tiles; Wv projection; multiply by 1/sums.
  - Wo from the transposed layout (lhsT = outT tiles), +query+bo, LN1,
    PE-transpose -> xlnT, FFN1 (f1T = W1.T @ xlnT, relu+bias fused on
    DVE), FFN2 (f2 = f1T.T @ W2), +bf2, +residual, LN3, DMA out.
"""

import sys

sys.path.insert(0, '/opt/trn_rl_repo')

import numpy as np

import concourse.bass as bass
import concourse.mybir as mybir
import concourse.tile as tile
from concourse.masks import make_identity

AF = mybir.ActivationFunctionType
ALU = mybir.AluOpType
F32 = mybir.dt.float32
F32R = mybir.dt.float32r

B, CN, CL, E, H, FF = 2, 16, 128, 1024, 16, 4096
HD = E // H               # 64
L = CN * CL               # 2048
KT = L // 128             # 16 kk tiles
PAIRS = H // 2            # 8 head pairs
Q = 512                   # query rows per core
QT = Q // 128             # 4
ET = E // 128             # 8
FT = FF // 128            # 32
EPS = 1e-5
SCALE = 1.0 / np.sqrt(E).astype(np.float32)  # note: sqrt(E), per reference
N_CORES = 8


# ---------------------------------------------------------------------------
# Walrus in this toolchain accepts only ONE sync wait per instruction:
# split any instruction carrying N>1 waits into N-1 single-wait NOPs on the
# same engine queue directly ahead of it.
# ---------------------------------------------------------------------------
def _split_block(nc, bb):
    insts = list(bb.instructions)
    out = []
    changed = False
    for inst in insts:
        si = inst.sync_info
        waits = list(si.on_wait) if si and si.on_wait else []
        if len(waits) > 1:
            changed = True
            for j, w in enumerate(waits[:-1]):
                nop = mybir.InstNoOp(
                    name=f"{inst.name}-w{j}",
                    engine=inst.engine,
                    bass_nofuse=True,
                    sync_info=mybir.SyncInfo(on_wait=[w], on_update=[]),
                )
                nc.register_instruction(nop, overwrite=True)
                out.append(nop)
            inst.sync_info = mybir.SyncInfo(
                on_wait=[waits[-1]], on_update=list(si.on_update or [])
            )
        out.append(inst)
    if changed:
        try:
            bb.instructions = out
        except Exception:
            del bb.instructions[:]
            bb.instructions.extend(out)
    for sub in getattr(bb, 'blocks', None) or []:
        _split_block(nc, sub)


class CompatTileContext(tile.TileContext):
    def schedule_and_allocate(self):
        r = super().schedule_and_allocate()
        for fn in self.nc.m.functions:
            for bb in fn.blocks:
                _split_block(self.nc, bb)
        return r

    def _drain_and_barrier(self, tick_clock, wait_clock):
        # Same as base, but clear sems in narrow chunks: this walrus
        # rejects wide EVENT_SEMAPHORE_RANGE_CLEAR ranges.
        from concourse.vector_clock import ScopedClock
        nc = self.nc
        drain_inst = nc.sync.drain()
        wait_clock.add_sem_waits(
            drain_inst.ins, ScopedClock({None: tick_clock.global_clock})
        )
        nc.all_engine_barrier()
        assert self.sems is not None
        popped = nc._tile_sem_poison_stack.pop()
        assert popped is self._sem_poison
        sems = list(self.sems.allocated().values())
        for i in range(0, len(sems), 2):
            nc.clear_and_free_semaphores(sems[i:i + 2])
        nc.all_engine_barrier()


def _bcast_ap(ap, parts):
    """Partition-broadcast AP: read a [N]/[1,N] source on `parts` partitions."""
    a = ap if len(ap.shape) > 1 else ap[None, :]
    return bass.AP(tensor=a.tensor, offset=a.offset, ap=[[0, parts], a.ap[-1]])


def build_nc(reps=1):
    nc = bass.Bass()

    v_aug = nc.dram_tensor("v_aug", [2, L, H * 65], F32R, kind="ExternalInput")
    qres = nc.dram_tensor("qres", [Q, E], F32, kind="ExternalInput")
    Wq_d = nc.dram_tensor("Wq", [HD, HD], F32, kind="ExternalInput")
    Wk_d = nc.dram_tensor("Wk", [HD, HD], F32, kind="ExternalInput")
    Wv_d = nc.dram_tensor("Wv", [HD, HD], F32, kind="ExternalInput")
    Wo_d = nc.dram_tensor("Wo_r", [128, ET, E], F32R, kind="ExternalInput")
    W1_d = nc.dram_tensor("W1_r", [128, ET, FF], F32R, kind="ExternalInput")
    W2_d = nc.dram_tensor("W2_r", [128, FT, E], F32R, kind="ExternalInput")
    bo_d = nc.dram_tensor("bo", [E], F32, kind="ExternalInput")
    g1_d = nc.dram_tensor("g1", [E], F32, kind="ExternalInput")
    b1_d = nc.dram_tensor("b1", [E], F32, kind="ExternalInput")
    bf1_d = nc.dram_tensor("bf1", [FF], F32, kind="ExternalInput")
    bf2_d = nc.dram_tensor("bf2", [E], F32, kind="ExternalInput")
    g3_d = nc.dram_tensor("g3", [E], F32, kind="ExternalInput")
    b3_d = nc.dram_tensor("b3", [E], F32, kind="ExternalInput")
    out_d = nc.dram_tensor("out", [Q, E], F32, kind="ExternalOutput")

    with CompatTileContext(nc) as tc:
        import contextlib
        with contextlib.ExitStack() as top:
            singles = top.enter_context(tc.tile_pool(name="singles", bufs=1))
            persist = top.enter_context(tc.tile_pool(name="persist", bufs=1))

            # --- constants / weights staged once ---
            ident_f = singles.tile([128, 128], F32)
            make_identity(nc, ident_f)
            ident_r = singles.tile([128, 128], F32R)
            nc.vector.tensor_copy(ident_r, ident_f)

            def blockdiag(dram_w, name):
                stg = singles.tile([128, 128], F32, tag=f"bd_{name}")
                nc.gpsimd.memset(stg, 0.0)
                nc.sync.dma_start(out=stg[0:HD, 0:HD], in_=dram_w[:, :])
                nc.sync.dma_start(out=stg[HD:128, HD:128], in_=dram_w[:, :])
                bd = singles.tile([128, 128], F32R, tag=f"bdr_{name}")
                nc.vector.tensor_copy(bd, stg)
                return bd

            Wq_bd = blockdiag(Wq_d, "q")
            Wk_bd = blockdiag(Wk_d, "k")

            Wv_stg = singles.tile([HD, HD], F32)
            nc.sync.dma_start(out=Wv_stg, in_=Wv_d[:, :])
            Wv_sb = singles.tile([HD, HD], F32R)
            nc.vector.tensor_copy(Wv_sb, Wv_stg)

            ones1_f = singles.tile([1, HD], F32)
            nc.vector.memset(ones1_f, 1.0)
            ones1 = singles.tile([1, HD], F32R)
            nc.vector.tensor_copy(ones1, ones1_f)

            def bcast(dram_ap, name):
                t = singles.tile([128, E], F32, tag=f"bc_{name}")
                nc.sync.dma_start(out=t, in_=_bcast_ap(dram_ap, 128))
                return t

            bo_bc = bcast(bo_d[:], "bo")
            g1_bc = bcast(g1_d[:], "g1")
            b1_bc = bcast(b1_d[:], "b1")
            bf2_bc = bcast(bf2_d[:], "bf2")
            g3_bc = bcast(g3_d[:], "g3")
            b3_bc = bcast(b3_d[:], "b3")

            bf1_sb = singles.tile([128, FT], F32)
            nc.sync.dma_start(
                out=bf1_sb, in_=bf1_d.rearrange("(t p) -> p t", p=128)
            )
            eps_sb = singles.tile([128, 1], F32)
            nc.vector.memset(eps_sb, EPS)

            outT_acc = persist.tile([128, PAIRS, Q], F32R, tag="outT")

            # ---------------- per-iteration body ----------------
            def layernorm(x_ap, g_bc, b_bc, out_ap, pool, tag):
                stats = pool.tile([128, 2, 6], F32, tag=f"st_{tag}", bufs=2)
                mv = pool.tile([128, 2], F32, tag=f"mv_{tag}", bufs=2)
                for i in range(2):
                    nc.vector.bn_stats(
                        out=stats[:, i, :], in_=x_ap[:, i * 512:(i + 1) * 512]
                    )
                nc.vector.bn_aggr(out=mv, in_=stats)
                lnv = pool.tile([128, 1], F32, tag=f"sd_{tag}", bufs=2)
                nc.scalar.activation(
                    out=lnv, in_=mv[:, 1:2], func=AF.Ln, bias=eps_sb
                )
                rstd = pool.tile([128, 1], F32, tag=f"rs_{tag}", bufs=2)
                nc.scalar.activation(
                    out=rstd, in_=lnv, func=AF.Exp, scale=-0.5
                )
                xn = pool.tile([128, E], F32, tag=f"xn_{tag}", bufs=1)
                nc.vector.tensor_scalar(
                    out=xn, in0=x_ap, scalar1=mv[:, 0:1], scalar2=rstd,
                    op0=ALU.subtract, op1=ALU.mult,
                )
                nc.vector.tensor_mul(xn, xn, g_bc)
                nc.vector.tensor_add(out_ap, xn, b_bc)

            # Per-core query-slice selection is handled host-side: each
            # core's v_aug rows are rotated so its 512 query rows are
            # always kk-chunk 0 (attention sums are order-invariant).
            def body():
                with contextlib.ExitStack() as span_ctx:
                    span = span_ctx.enter_context(
                        tc.tile_pool(name="span", bufs=1)
                    )
                    xln = span.tile([128, QT, E], F32, tag="xln")
                    xlnT = span.tile([128, ET, Q], F32R, tag="xlnT")

                    # ======== attention ========
                    with contextlib.ExitStack() as sA:
                        pa = sA.enter_context(tc.tile_pool(name="attn_sb", bufs=2))
                        pp = sA.enter_context(
                            tc.tile_pool(name="attn_ps", bufs=2, space="PSUM")
                        )
                        for vi in range(2):
                            x_aug = pa.tile([128, KT, H * 65], F32R,
                                            tag="xaug", bufs=1)
                            nc.sync.dma_start(
                                out=x_aug,
                                in_=v_aug[vi].rearrange(
                                    "(kt p) c -> p kt c", p=128),
                            )
                            for pair in range(PAIRS):
                                kT = pa.tile([128, L], F32R, tag="kT", bufs=2)
                                qT = pa.tile([128, Q], F32R, tag="qT", bufs=2)
                                # contiguous copy of this pair's two head
                                # blocks (moving operands need 1 free dim)
                                xp = pa.tile([128, KT, 2, HD], F32R,
                                             tag="xp", bufs=1)
                                nc.vector.tensor_copy(
                                    xp,
                                    x_aug.rearrange(
                                        "p kt (h c) -> p kt h c", c=65
                                    )[:, :, 2 * pair:2 * pair + 2, 0:HD])
                                for chunk in range(4):
                                    xTb = pa.tile([128, 512], F32R,
                                                  tag="xTb", bufs=2)
                                    for t in range(4):
                                        kkt = chunk * 4 + t
                                        tp = pp.tile([128, 128], F32R,
                                                     tag="tp", bufs=2)
                                        src = xp[:, kkt, :, :].rearrange(
                                            "p h c -> p (h c)")
                                        nc.tensor.transpose(tp, src, ident_r)
                                        nc.vector.tensor_copy(
                                            xTb[:, t * 128:(t + 1) * 128], tp)
                                    kps = pp.tile([128, 512], F32,
                                                  tag="kq", bufs=2)
                                    nc.tensor.matmul(kps, Wk_bd, xTb,
                                                     start=True, stop=True)
                                    nc.vector.tensor_copy(
                                        kT[:, chunk * 512:(chunk + 1) * 512],
                                        kps)
                                    if chunk == 0:
                                        qps = pp.tile([128, 512], F32,
                                                      tag="kq", bufs=2)
                                        nc.tensor.matmul(qps, Wq_bd, xTb,
                                                         start=True, stop=True)
                                        nc.vector.tensor_copy(qT, qps)
                                for hi in range(2):
                                    h = 2 * pair + hi
                                    prow = slice(64 * hi, 64 * hi + 64)
                                    av = pp.tile([65, 512], F32,
                                                 tag="av", bufs=1)
                                    for eb in range(KT // 2):
                                        e_ps = pp.tile([128, 1024], F32,
                                                       tag="e", bufs=1)
                                        for j in range(2):
                                            kkt = eb * 2 + j
                                            nc.tensor.matmul(
                                                e_ps[:, j * 512:(j + 1) * 512],
                                                kT[prow,
                                                   kkt * 128:(kkt + 1) * 128],
                                                qT[prow, :],
                                                start=True, stop=True)
                                        att = pa.tile([128, 1024], F32R,
                                                      tag="att", bufs=2)
                                        nc.scalar.activation(
                                            att, e_ps, AF.Exp, scale=SCALE)
                                        for j in range(2):
                                            kkt = eb * 2 + j
                                            nc.tensor.matmul(
                                                av,
                                                x_aug[:, kkt,
                                                      h * 65:(h + 1) * 65],
                                                att[:, j * 512:(j + 1) * 512],
                                                start=(kkt == 0),
                                                stop=(kkt == KT - 1))
                                    # softmax normalizer: r = 1/sums
                                    r1a = pa.tile([1, 512], F32,
                                                  tag="r1", bufs=1)
                                    nc.scalar.activation(
                                        out=r1a, in_=av[64:65, :], func=AF.Ln)
                                    r1b = pa.tile([1, 512], F32R,
                                                  tag="r1b", bufs=2)
                                    nc.scalar.activation(
                                        out=r1b, in_=r1a, func=AF.Exp,
                                        scale=-1.0)
                                    out1 = pa.tile([64, 512], F32R,
                                                   tag="out1", bufs=2)
                                    nc.vector.tensor_copy(out1, av[0:64, :])
                                    # wv reuses the av bank (av fully consumed)
                                    wv_ps = pp.tile([64, 512], F32,
                                                    tag="av", bufs=1)
                                    nc.tensor.matmul(wv_ps, Wv_sb, out1,
                                                     start=True, stop=True)
                                    # partition-broadcast of 1/sums via
                                    # rank-1 matmul: ones[1,64].T @ r1b[1,q]
                                    rbc_ps = pp.tile([64, 512], F32,
                                                     tag="rbc", bufs=1)
                                    nc.tensor.matmul(rbc_ps, ones1, r1b,
                                                     start=True, stop=True)
                                    r_bc = pa.tile([64, 512], F32,
                                                   tag="rbc_sb", bufs=1)
                                    nc.vector.tensor_copy(r_bc, rbc_ps)
                                    dst = outT_acc[prow, pair, :]
                                    if hi == 0:
                                        if vi == 0:
                                            nc.vector.tensor_mul(
                                                dst, wv_ps, r_bc)
                                        else:
                                            tmp = pa.tile([64, 512], F32,
                                                          tag="nrm", bufs=2)
                                            nc.vector.tensor_mul(
                                                tmp, wv_ps, r_bc)
                                            nc.vector.tensor_add(
                                                dst, dst, tmp)
                                    else:
                                        # rows 64-127: DVE lanes can't shift
                                        # partitions; normalize at base 0 and
                                        # DMA into place.
                                        norm = pa.tile([64, 512], F32R,
                                                       tag="nrm2", bufs=2)
                                        nc.vector.tensor_mul(
                                            norm, wv_ps, r_bc)
                                        if vi == 0:
                                            nc.sync.dma_start(
                                                out=dst, in_=norm)
                                        else:
                                            st64 = pa.tile([128, 512], F32R,
                                                           tag="st64", bufs=2)
                                            nc.sync.dma_start(
                                                out=st64[64:128, :], in_=norm)
                                            nc.vector.tensor_add(
                                                dst, dst, st64[64:128, :])

                    # ======== Wo + residual + LN1 + transpose ========
                    with contextlib.ExitStack() as sB:
                        pb = sB.enter_context(tc.tile_pool(name="wo_sb", bufs=2))
                        pq = sB.enter_context(
                            tc.tile_pool(name="wo_ps", bufs=2, space="PSUM"))
                        Wo_sb = pb.tile([128, ET, E], F32R, tag="wow", bufs=1)
                        nc.sync.dma_start(out=Wo_sb, in_=Wo_d[:, :, :])
                        for qs in range(QT):
                            q_t = pb.tile([128, E], F32, tag="qt", bufs=2)
                            nc.sync.dma_start(
                                out=q_t, in_=qres[qs * 128:(qs + 1) * 128, :])
                            xr = pb.tile([128, E], F32, tag="xr", bufs=2)
                            for eh in range(2):
                                sl = slice(eh * 512, (eh + 1) * 512)
                                wo_ps = pq.tile([128, 512], F32,
                                                tag="wo", bufs=4)
                                for pair in range(PAIRS):
                                    nc.tensor.matmul(
                                        wo_ps,
                                        outT_acc[:, pair,
                                                 qs * 128:(qs + 1) * 128],
                                        Wo_sb[:, pair, sl],
                                        start=(pair == 0),
                                        stop=(pair == PAIRS - 1))
                                nc.vector.tensor_add(xr[:, sl], wo_ps,
                                                     q_t[:, sl])
                                nc.vector.tensor_add(xr[:, sl], xr[:, sl],
                                                     bo_bc[:, sl])
                            layernorm(xr, g1_bc, b1_bc, xln[:, qs, :],
                                      pb, "ln1")
                            for et in range(ET):
                                tp = pq.tile([128, 128], F32,
                                             tag="tpx", bufs=2)
                                nc.tensor.transpose(
                                    tp,
                                    xln[:, qs, et * 128:(et + 1) * 128],
                                    ident_f)
                                nc.vector.tensor_copy(
                                    xlnT[:, et, qs * 128:(qs + 1) * 128], tp)

                    # ======== FFN ========
                    with contextlib.ExitStack() as sF:
                        pf = sF.enter_context(
                            tc.tile_pool(name="ffn_span", bufs=1))
                        f1T = pf.tile([128, FT, Q], F32R, tag="f1T")
                        sC = contextlib.ExitStack()
                        pc = sC.enter_context(tc.tile_pool(name="f1_sb", bufs=2))
                        pr = sC.enter_context(
                            tc.tile_pool(name="f1_ps", bufs=2, space="PSUM"))
                        for g in range(8):
                            w1c = pc.tile([128, ET, 512], F32R,
                                          tag="w1", bufs=2)
                            nc.sync.dma_start(
                                out=w1c,
                                in_=W1_d[:, :, g * 512:(g + 1) * 512])
                            for t in range(4):
                                ft = g * 4 + t
                                f1ps = pr.tile([128, 512], F32,
                                               tag="f1", bufs=4)
                                for et in range(ET):
                                    nc.tensor.matmul(
                                        f1ps,
                                        w1c[:, et, t * 128:(t + 1) * 128],
                                        xlnT[:, et, :],
                                        start=(et == 0), stop=(et == ET - 1))
                                nc.vector.tensor_scalar(
                                    out=f1T[:, ft, :], in0=f1ps,
                                    scalar1=bf1_sb[:, ft:ft + 1], scalar2=0.0,
                                    op0=ALU.add, op1=ALU.max)

                        sC.close()
                        # ==== FFN2 + bf2 + residual + LN3 + out ====
                        pd = sF.enter_context(tc.tile_pool(name="f2_sb", bufs=2))
                        pt2 = sF.enter_context(
                            tc.tile_pool(name="f2_ps", bufs=2, space="PSUM"))
                        f2ps = [pt2.tile([128, 512], F32, tag="f2", bufs=8,
                                         name=f"f2ps{i}")
                                for i in range(8)]
                        for ft in range(FT):
                            w2c = pd.tile([128, E], F32R, tag="w2", bufs=3)
                            nc.sync.dma_start(out=w2c, in_=W2_d[:, ft, :])
                            for qs in range(QT):
                                for eh in range(2):
                                    nc.tensor.matmul(
                                        f2ps[qs * 2 + eh],
                                        f1T[:, ft, qs * 128:(qs + 1) * 128],
                                        w2c[:, eh * 512:(eh + 1) * 512],
                                        start=(ft == 0), stop=(ft == FT - 1))
                        for qs in range(QT):
                            y = pd.tile([128, E], F32, tag="y", bufs=2)
                            for eh in range(2):
                                sl = slice(eh * 512, (eh + 1) * 512)
                                nc.vector.tensor_add(
                                    y[:, sl], f2ps[qs * 2 + eh], bf2_bc[:, sl])
                            nc.vector.tensor_add(y, y, xln[:, qs, :])
                            out_sb = pd.tile([128, E], F32, tag="ysb", bufs=2)
                            layernorm(y, g3_bc, b3_bc, out_sb, pd, "ln3")
                            nc.sync.dma_start(
                                out=out_d[qs * 128:(qs + 1) * 128, :],
                                in_=out_sb)

            if reps == 1:
                body()
            else:
                with tc.For_i(0, reps, 1):
                    body()

    return nc


# ---------------------------------------------------------------------------
# Host side: compile-once runner over PJRT (axon), sharding, gather.
# ---------------------------------------------------------------------------
def _make_runner(nc, n_cores):
    import time

    import jax
    from jax.experimental.shard_map import shard_map
    from jax.sharding import Mesh, PartitionSpec

    from concourse.bass2jax import (
        _bass_exec_p,
        install_neuronx_cc_hook,
        partition_id_tensor,
    )

    install_neuronx_cc_hook()
    partition_name = (
        nc.partition_id_tensor.name if nc.partition_id_tensor else None
    )

    in_names, out_names, out_avals, zero_outs = [], [], [], []
    for alloc in nc.m.functions[0].allocations:
        if not isinstance(alloc, mybir.MemoryLocationSet):
            continue
        name = alloc.memorylocations[0].name
        if alloc.kind == "ExternalInput":
            if name != partition_name:
                in_names.append(name)
        elif alloc.kind == "ExternalOutput":
            shape = tuple(alloc.tensor_shape)
            dtype = mybir.dt.np(alloc.dtype)
            out_names.append(name)
            out_avals.append(jax.core.ShapedArray(shape, dtype))
            zero_outs.append(np.zeros(shape, dtype))
    n_params = len(in_names)
    n_outs = len(out_avals)
    all_in_names = list(in_names) + list(out_names)
    if partition_name is not None:
        all_in_names.append(partition_name)
    donate = tuple(range(n_params, n_params + n_outs))

    def _body(*args):
        operands = list(args)
        if partition_name is not None:
            operands.append(partition_id_tensor())
        outs = _bass_exec_p.bind(
            *operands,
            out_avals=tuple(out_avals),
            in_names=tuple(all_in_names),
            out_names=tuple(out_names),
            lowering_input_output_aliases=(),
            sim_require_finite=True,
            sim_require_nnan=True,
            nc=nc,
        )
        return tuple(outs)

    devices = jax.devices()[:n_cores]
    assert len(devices) == n_cores, f"need {n_cores} cores, saw {len(jax.devices())}"
    mesh = Mesh(np.asarray(devices), ("core",))
    sharded = jax.jit(
        shard_map(
            _body, mesh=mesh,
            in_specs=(PartitionSpec("core"),) * (n_params + n_outs),
            out_specs=(PartitionSpec("core"),) * n_outs,
            check_rep=False,
        ),
        donate_argnums=donate,
        keep_unused=True,
    )

    def run(in_maps, time_reps=0):
        per_core = [[np.asarray(m[name]) for name in in_names]
                    for m in in_maps]
        concat_in = [
            np.concatenate([per_core[c][i] for c in range(n_cores)], axis=0)
            for i in range(n_params)
        ]
        def zeros():
            return [np.zeros((n_cores * z.shape[0], *z.shape[1:]), z.dtype)
                    for z in zero_outs]
        out_arrs = jax.block_until_ready(sharded(*concat_in, *zeros()))
        results = [
            {n: np.asarray(out_arrs[i]).reshape(n_cores,
                                                *out_avals[i].shape)[c]
             for i, n in enumerate(out_names)}
            for c in range(n_cores)
        ]
        times = []
        if time_reps:
            from jax.sharding import NamedSharding
            dev_in = [
                jax.device_put(a, NamedSharding(mesh, PartitionSpec("core")))
                for a in concat_in
            ]
            for _ in range(time_reps):
                z = [jax.device_put(
                        zz, NamedSharding(mesh, PartitionSpec("core")))
                     for zz in zeros()]
                jax.block_until_ready(z)
                t0 = time.perf_counter()
                jax.block_until_ready(sharded(*dev_in, *z))
                times.append(time.perf_counter() - t0)
        return results, times

    return run


_cache = {}


def _get_runner(reps=1):
    key = ("runner", reps)
    if key not in _cache:
        nc = build_nc(reps=reps)
        _cache[key] = _make_runner(nc, N_CORES)
    return _cache[key]


def _shard_inputs(value, query, Wv, Wk, Wq, Wo, bo, g1, b1, W1, bf1, W2,
                  bf2, g3, b3):
    f = lambda a: np.ascontiguousarray(np.asarray(a, np.float32))
    value, query = f(value), f(query)
    Wo_r = np.ascontiguousarray(
        f(Wo).reshape(ET, 128, E).transpose(1, 0, 2))
    W1_r = np.ascontiguousarray(
        f(W1).reshape(ET, 128, FF).transpose(1, 0, 2))
    W2_r = np.ascontiguousarray(
        f(W2).reshape(FT, 128, E).transpose(1, 0, 2))
    shared = {
        "Wq": f(Wq), "Wk": f(Wk), "Wv": f(Wv),
        "Wo_r": Wo_r, "W1_r": W1_r, "W2_r": W2_r,
        "bo": f(bo), "g1": f(g1), "b1": f(b1), "bf1": f(bf1),
        "bf2": f(bf2), "g3": f(g3), "b3": f(b3),
    }
    in_maps = []
    for b in range(B):
        va = np.empty((2, L, H, 65), np.float32)
        for vi in range(2):
            va[vi, :, :, :HD] = value[b, :, vi].reshape(L, H, HD)
            va[vi, :, :, HD] = 1.0
        va_b = va.reshape(2, L, H * 65)
        qb = query[b].reshape(L, E)
        for qs in range(4):
            in_maps.append({
                "v_aug": np.ascontiguousarray(
                    np.roll(va_b, -qs * Q, axis=1)),
                "qres": np.ascontiguousarray(qb[qs * Q:(qs + 1) * Q]),
                **shared,
            })
    return in_maps


def kernel(value, key, query, mask, retrieved_passages,
           Wv, Wk, Wq, Wo, bo, g1, b1, W1, bf1, W2, bf2, g3, b3):
    in_maps = _shard_inputs(value, query, Wv, Wk, Wq, Wo, bo, g1, b1,
                            W1, bf1, W2, bf2, g3, b3)
    run = _get_runner(reps=1)
    results, _ = run(in_maps, 0)
    out = np.empty((B, L, E), np.float32)
    for b in range(B):
        for qs in range(4):
            out[b, qs * Q:(qs + 1) * Q] = results[b * 4 + qs]["out"]
    return out.reshape(B, CN, CL, E)



# revision 19
# speedup vs baseline: 1.7905x; 1.7905x over previous
"""Trainium2 Bass kernel for nn_EncoderLayer_11132555231236.

Computation (reference.py): two self-attentions over value[:, :, 0/1]
(Q/K/V all derived from the value tensor via shared per-head 64x64
projections), summed, +query residual, LN, FFN(1024->4096->1024), +res, LN.

Sharding: 8 cores = (batch b in {0,1}) x (query-row slice qs in {0..3},
512 rows each of L=2048). No collectives: each core computes K/V for the
full sequence of its batch (cheap per-head 64x64 projections) and its own
512 query rows end-to-end through the FFN.

Dataflow per core (all matmuls in float32r = TF32-rate, fp32 accumulate):
  - x_aug = [x | ones] DMA'd per head-block (ones column yields softmax
    sums for free via the att.T @ x_aug matmul).
  - PE-transpose x blocks -> xT per head-pair; kT = blockdiag(Wk,Wk).T@xT,
    qT likewise (2 heads per 128-contraction matmul).
  - energyT[kk, q] = kT.T @ qT per 128-kk tile; exp on ScalarE (scale
    1/sqrt(E)); unnormalized out1T[c,q] (+sums row) = x_aug.T @ attT
    accumulated over kk tiles; Wv projection; multiply by 1/sums.
  - Wo from the transposed layout (lhsT = outT tiles), +query+bo, LN1,
    PE-transpose -> xlnT, FFN1 (f1T = W1.T @ xlnT, relu+bias fused on
    DVE), FFN2 (f2 = f1T.T @ W2), +bf2, +residual, LN3, DMA out.
"""

import sys

sys.path.insert(0, '/opt/trn_rl_repo')

import numpy as np

import concourse.bass as bass
import concourse.mybir as mybir
import concourse.tile as tile
from concourse.masks import make_identity

AF = mybir.ActivationFunctionType
ALU = mybir.AluOpType
F32 = mybir.dt.float32
F32R = mybir.dt.float32r

B, CN, CL, E, H, FF = 2, 16, 128, 1024, 16, 4096
HD = E // H               # 64
L = CN * CL               # 2048
KT = L // 128             # 16 kk tiles
PAIRS = H // 2            # 8 head pairs
Q = 512                   # query rows per core
QT = Q // 128             # 4
ET = E // 128             # 8
FT = FF // 128            # 32
EPS = 1e-5
SCALE = 1.0 / np.sqrt(E).astype(np.float32)  # note: sqrt(E), per reference
N_CORES = 8


# ---------------------------------------------------------------------------
# Walrus in this toolchain accepts only ONE sync wait per instruction:
# split any instruction carrying N>1 waits into N-1 single-wait NOPs on the
# same engine queue directly ahead of it.
# ---------------------------------------------------------------------------
def _split_block(nc, bb):
    insts = list(bb.instructions)
    out = []
    changed = False
    for inst in insts:
        si = inst.sync_info
        waits = list(si.on_wait) if si and si.on_wait else []
        if len(waits) > 1:
            changed = True
            for j, w in enumerate(waits[:-1]):
                nop = mybir.InstNoOp(
                    name=f"{inst.name}-w{j}",
                    engine=inst.engine,
                    bass_nofuse=True,
                    sync_info=mybir.SyncInfo(on_wait=[w], on_update=[]),
                )
                nc.register_instruction(nop, overwrite=True)
                out.append(nop)
            inst.sync_info = mybir.SyncInfo(
                on_wait=[waits[-1]], on_update=list(si.on_update or [])
            )
        out.append(inst)
    if changed:
        try:
            bb.instructions = out
        except Exception:
            del bb.instructions[:]
            bb.instructions.extend(out)
    for sub in getattr(bb, 'blocks', None) or []:
        _split_block(nc, sub)


class CompatTileContext(tile.TileContext):
    def schedule_and_allocate(self):
        r = super().schedule_and_allocate()
        for fn in self.nc.m.functions:
            for bb in fn.blocks:
                _split_block(self.nc, bb)
        return r

    def _drain_and_barrier(self, tick_clock, wait_clock):
        # Same as base, but clear sems in narrow chunks: this walrus
        # rejects wide EVENT_SEMAPHORE_RANGE_CLEAR ranges.
        from concourse.vector_clock import ScopedClock
        nc = self.nc
        drain_inst = nc.sync.drain()
        wait_clock.add_sem_waits(
            drain_inst.ins, ScopedClock({None: tick_clock.global_clock})
        )
        nc.all_engine_barrier()
        assert self.sems is not None
        popped = nc._tile_sem_poison_stack.pop()
        assert popped is self._sem_poison
        sems = list(self.sems.allocated().values())
        for i in range(0, len(sems), 2):
            nc.clear_and_free_semaphores(sems[i:i + 2])
        nc.all_engine_barrier()


def _bcast_ap(ap, parts):
    """Partition-broadcast AP: read a [N]/[1,N] source on `parts` partitions."""
    a = ap if len(ap.shape) > 1 else ap[None, :]
    return bass.AP(tensor=a.tensor, offset=a.offset, ap=[[0, parts], a.ap[-1]])


def build_nc(reps=1):
    nc = bass.Bass()

    v_aug = nc.dram_tensor("v_aug", [2, L, H * 65], F32R, kind="ExternalInput")
    qres = nc.dram_tensor("qres", [Q, E], F32, kind="ExternalInput")
    Wq_d = nc.dram_tensor("Wq", [HD, HD], F32, kind="ExternalInput")
    Wk_d = nc.dram_tensor("Wk", [HD, HD], F32, kind="ExternalInput")
    Wv_d = nc.dram_tensor("Wv", [HD, HD], F32, kind="ExternalInput")
    Wo_d = nc.dram_tensor("Wo_r", [128, ET, E], F32R, kind="ExternalInput")
    W1_d = nc.dram_tensor("W1_r", [128, ET, FF], F32R, kind="ExternalInput")
    W2_d = nc.dram_tensor("W2_r", [128, FT, E], F32R, kind="ExternalInput")
    bo_d = nc.dram_tensor("bo", [E], F32, kind="ExternalInput")
    g1_d = nc.dram_tensor("g1", [E], F32, kind="ExternalInput")
    b1_d = nc.dram_tensor("b1", [E], F32, kind="ExternalInput")
    bf1_d = nc.dram_tensor("bf1", [FF], F32, kind="ExternalInput")
    bf2_d = nc.dram_tensor("bf2", [E], F32, kind="ExternalInput")
    g3_d = nc.dram_tensor("g3", [E], F32, kind="ExternalInput")
    b3_d = nc.dram_tensor("b3", [E], F32, kind="ExternalInput")
    out_d = nc.dram_tensor("out", [Q, E], F32, kind="ExternalOutput")

    with CompatTileContext(nc) as tc:
        import contextlib
        with contextlib.ExitStack() as top:
            singles = top.enter_context(tc.tile_pool(name="singles", bufs=1))
            persist = top.enter_context(tc.tile_pool(name="persist", bufs=1))

            # --- constants / weights staged once ---
            ident_f = singles.tile([128, 128], F32)
            make_identity(nc, ident_f)
            ident_r = singles.tile([128, 128], F32R)
            nc.vector.tensor_copy(ident_r, ident_f)

            def blockdiag(dram_w, name):
                stg = singles.tile([128, 128], F32, tag=f"bd_{name}")
                nc.gpsimd.memset(stg, 0.0)
                nc.sync.dma_start(out=stg[0:HD, 0:HD], in_=dram_w[:, :])
                nc.sync.dma_start(out=stg[HD:128, HD:128], in_=dram_w[:, :])
                bd = singles.tile([128, 128], F32R, tag=f"bdr_{name}")
                nc.vector.tensor_copy(bd, stg)
                return bd

            Wq_bd = blockdiag(Wq_d, "q")
            Wk_bd = blockdiag(Wk_d, "k")

            Wv_stg = singles.tile([HD, HD], F32)
            nc.sync.dma_start(out=Wv_stg, in_=Wv_d[:, :])
            Wv_sb = singles.tile([HD, HD], F32R)
            nc.vector.tensor_copy(Wv_sb, Wv_stg)

            ones1_f = singles.tile([1, HD], F32)
            nc.vector.memset(ones1_f, 1.0)
            ones1 = singles.tile([1, HD], F32R)
            nc.vector.tensor_copy(ones1, ones1_f)

            def bcast(dram_ap, name):
                t = singles.tile([128, E], F32, tag=f"bc_{name}")
                nc.sync.dma_start(out=t, in_=_bcast_ap(dram_ap, 128))
                return t

            bo_bc = bcast(bo_d[:], "bo")
            g1_bc = bcast(g1_d[:], "g1")
            b1_bc = bcast(b1_d[:], "b1")
            bf2_bc = bcast(bf2_d[:], "bf2")
            g3_bc = bcast(g3_d[:], "g3")
            b3_bc = bcast(b3_d[:], "b3")

            bf1_sb = singles.tile([128, FT], F32)
            nc.sync.dma_start(
                out=bf1_sb, in_=bf1_d.rearrange("(t p) -> p t", p=128)
            )
            eps_sb = singles.tile([128, 1], F32)
            nc.vector.memset(eps_sb, EPS)

            outT_acc = persist.tile([128, PAIRS, Q], F32R, tag="outT")

            # ---------------- per-iteration body ----------------
            def layernorm(x_ap, g_bc, b_bc, out_ap, pool, tag):
                stats = pool.tile([128, 2, 6], F32, tag=f"st_{tag}", bufs=2)
                mv = pool.tile([128, 2], F32, tag=f"mv_{tag}", bufs=2)
                for i in range(2):
                    nc.vector.bn_stats(
                        out=stats[:, i, :], in_=x_ap[:, i * 512:(i + 1) * 512]
                    )
                nc.vector.bn_aggr(out=mv, in_=stats)
                lnv = pool.tile([128, 1], F32, tag=f"sd_{tag}", bufs=2)
                nc.scalar.activation(
                    out=lnv, in_=mv[:, 1:2], func=AF.Ln, bias=eps_sb
                )
                rstd = pool.tile([128, 1], F32, tag=f"rs_{tag}", bufs=2)
                nc.scalar.activation(
                    out=rstd, in_=lnv, func=AF.Exp, scale=-0.5
                )
                xn = pool.tile([128, E], F32, tag=f"xn_{tag}", bufs=1)
                nc.vector.tensor_scalar(
                    out=xn, in0=x_ap, scalar1=mv[:, 0:1], scalar2=rstd,
                    op0=ALU.subtract, op1=ALU.mult,
                )
                nc.vector.tensor_mul(xn, xn, g_bc)
                nc.vector.tensor_add(out_ap, xn, b_bc)

            # Per-core query-slice selection is handled host-side: each
            # core's v_aug rows are rotated so its 512 query rows are
            # always kk-chunk 0 (attention sums are order-invariant).
            def body():
                with contextlib.ExitStack() as span_ctx:
                    span = span_ctx.enter_context(
                        tc.tile_pool(name="span", bufs=1)
                    )
                    xln = span.tile([128, QT, E], F32, tag="xln")
                    xlnT = span.tile([128, ET, Q], F32R, tag="xlnT")

                    # ======== attention ========
                    with contextlib.ExitStack() as sA:
                        pa = sA.enter_context(tc.tile_pool(name="attn_sb", bufs=2))
                        pp = sA.enter_context(
                            tc.tile_pool(name="attn_ps", bufs=2, space="PSUM")
                        )
                        for vi in range(2):
                            x_aug = pa.tile([128, KT, H * 65], F32R,
                                            tag="xaug", bufs=1)
                            v_r = v_aug[vi].rearrange(
                                "(kt p) c -> p kt c", p=128)
                            for dc in range(4):
                                nc.sync.dma_start(
                                    out=x_aug[:, dc * 4:(dc + 1) * 4, :],
                                    in_=v_r[:, dc * 4:(dc + 1) * 4, :],
                                )
                            for pair in range(PAIRS):
                                kT = pa.tile([128, L], F32R, tag="kT", bufs=2)
                                qT = pa.tile([128, Q], F32R, tag="qT", bufs=2)
                                # contiguous copy of this pair's two head
                                # blocks (moving operands need 1 free dim)
                                xp = pa.tile([128, KT, 2, HD], F32R,
                                             tag="xp", bufs=1)
                                nc.vector.tensor_copy(
                                    xp,
                                    x_aug.rearrange(
                                        "p kt (h c) -> p kt h c", c=65
                                    )[:, :, 2 * pair:2 * pair + 2, 0:HD])
                                for chunk in range(4):
                                    xTb = pa.tile([128, 512], F32R,
                                                  tag="xTb", bufs=2)
                                    tp = pp.tile([128, 512], F32R,
                                                 tag="tpkq", bufs=2)
                                    for t in range(4):
                                        kkt = chunk * 4 + t
                                        src = xp[:, kkt, :, :].rearrange(
                                            "p h c -> p (h c)")
                                        nc.tensor.transpose(
                                            tp[:, t * 128:(t + 1) * 128],
                                            src, ident_r)
                                    nc.vector.tensor_copy(xTb, tp)
                                    kps = pp.tile([128, 512], F32,
                                                  tag="tpkq", bufs=2)
                                    nc.tensor.matmul(kps, Wk_bd, xTb,
                                                     start=True, stop=True)
                                    nc.vector.tensor_copy(
                                        kT[:, chunk * 512:(chunk + 1) * 512],
                                        kps)
                                    if chunk == 0:
                                        qps = pp.tile([128, 512], F32,
                                                      tag="tpkq", bufs=2)
                                        nc.tensor.matmul(qps, Wq_bd, xTb,
                                                         start=True, stop=True)
                                        nc.vector.tensor_copy(qT, qps)
                                for hi in range(2):
                                    h = 2 * pair + hi
                                    prow = slice(64 * hi, 64 * hi + 64)
                                    av = pp.tile([65, 512], F32,
                                                 tag="av", bufs=1)
                                    for eb in range(KT // 2):
                                        e_ps = pp.tile([128, 1024], F32,
                                                       tag="e", bufs=2)
                                        for j in range(2):
                                            kkt = eb * 2 + j
                                            nc.tensor.matmul(
                                                e_ps[:, j * 512:(j + 1) * 512],
                                                kT[prow,
                                                   kkt * 128:(kkt + 1) * 128],
                                                qT[prow, :],
                                                start=True, stop=True)
                                        att = pa.tile([128, 1024], F32R,
                                                      tag="att", bufs=2)
                                        nc.scalar.activation(
                                            att, e_ps, AF.Exp, scale=SCALE)
                                        for j in range(2):
                                            kkt = eb * 2 + j
                                            nc.tensor.matmul(
                                                av,
                                                x_aug[:, kkt,
                                                      h * 65:(h + 1) * 65],
                                                att[:, j * 512:(j + 1) * 512],
                                                start=(kkt == 0),
                                                stop=(kkt == KT - 1))
                                    # softmax normalizer: r = 1/sums
                                    r1a = pa.tile([1, 512], F32,
                                                  tag="r1", bufs=1)
                                    nc.scalar.activation(
                                        out=r1a, in_=av[64:65, :], func=AF.Ln)
                                    r1b = pa.tile([1, 512], F32R,
                                                  tag="r1b", bufs=2)
                                    nc.scalar.activation(
                                        out=r1b, in_=r1a, func=AF.Exp,
                                        scale=-1.0)
                                    out1 = pa.tile([64, 512], F32R,
                                                   tag="out1", bufs=2)
                                    nc.vector.tensor_copy(out1, av[0:64, :])
                                    # wv reuses the av bank (av fully consumed)
                                    wv_ps = pp.tile([64, 512], F32,
                                                    tag="av", bufs=1)
                                    nc.tensor.matmul(wv_ps, Wv_sb, out1,
                                                     start=True, stop=True)
                                    # partition-broadcast of 1/sums via
                                    # rank-1 matmul: ones[1,64].T @ r1b[1,q]
                                    rbc_ps = pp.tile([64, 512], F32,
                                                     tag="rbc", bufs=1)
                                    nc.tensor.matmul(rbc_ps, ones1, r1b,
                                                     start=True, stop=True)
                                    r_bc = pa.tile([64, 512], F32,
                                                   tag="rbc_sb", bufs=1)
                                    nc.vector.tensor_copy(r_bc, rbc_ps)
                                    dst = outT_acc[prow, pair, :]
                                    if hi == 0:
                                        if vi == 0:
                                            nc.vector.tensor_mul(
                                                dst, wv_ps, r_bc)
                                        else:
                                            tmp = pa.tile([64, 512], F32,
                                                          tag="nrm", bufs=2)
                                            nc.vector.tensor_mul(
                                                tmp, wv_ps, r_bc)
                                            nc.vector.tensor_add(
                                                dst, dst, tmp)
                                    else:
                                        # rows 64-127: DVE lanes can't shift
                                        # partitions; normalize at base 0 and
                                        # DMA into place.
                                        norm = pa.tile([64, 512], F32R,
                                                       tag="nrm2", bufs=2)
                                        nc.vector.tensor_mul(
                                            norm, wv_ps, r_bc)
                                        if vi == 0:
                                            nc.sync.dma_start(
                                                out=dst, in_=norm)
                                        else:
                                            st64 = pa.tile([128, 512], F32R,
                                                           tag="st64", bufs=2)
                                            nc.sync.dma_start(
                                                out=st64[64:128, :], in_=norm)
                                            nc.vector.tensor_add(
                                                dst, dst, st64[64:128, :])

                    # ======== Wo + residual + LN1 + transpose ========
                    with contextlib.ExitStack() as sB:
                        pb = sB.enter_context(tc.tile_pool(name="wo_sb", bufs=2))
                        pq = sB.enter_context(
                            tc.tile_pool(name="wo_ps", bufs=2, space="PSUM"))
                        Wo_sb = pb.tile([128, ET, E], F32R, tag="wow", bufs=1)
                        nc.sync.dma_start(out=Wo_sb, in_=Wo_d[:, :, :])
                        for qs in range(QT):
                            q_t = pb.tile([128, E], F32, tag="qt", bufs=2)
                            nc.sync.dma_start(
                                out=q_t, in_=qres[qs * 128:(qs + 1) * 128, :])
                            xr = pb.tile([128, E], F32, tag="xr", bufs=2)
                            for eh in range(2):
                                sl = slice(eh * 512, (eh + 1) * 512)
                                wo_ps = pq.tile([128, 512], F32,
                                                tag="wo", bufs=4)
                                for pair in range(PAIRS):
                                    nc.tensor.matmul(
                                        wo_ps,
                                        outT_acc[:, pair,
                                                 qs * 128:(qs + 1) * 128],
                                        Wo_sb[:, pair, sl],
                                        start=(pair == 0),
                                        stop=(pair == PAIRS - 1))
                                nc.vector.tensor_add(xr[:, sl], wo_ps,
                                                     q_t[:, sl])
                                nc.vector.tensor_add(xr[:, sl], xr[:, sl],
                                                     bo_bc[:, sl])
                            layernorm(xr, g1_bc, b1_bc, xln[:, qs, :],
                                      pb, "ln1")
                            for et in range(ET):
                                tp = pq.tile([128, 128], F32,
                                             tag="tpx", bufs=2)
                                nc.tensor.transpose(
                                    tp,
                                    xln[:, qs, et * 128:(et + 1) * 128],
                                    ident_f)
                                nc.vector.tensor_copy(
                                    xlnT[:, et, qs * 128:(qs + 1) * 128], tp)

                    # ======== FFN ========
                    with contextlib.ExitStack() as sF:
                        pf = sF.enter_context(
                            tc.tile_pool(name="ffn_span", bufs=1))
                        f1T = pf.tile([128, FT, Q], F32R, tag="f1T")
                        sC = contextlib.ExitStack()
                        pc = sC.enter_context(tc.tile_pool(name="f1_sb", bufs=2))
                        pr = sC.enter_context(
                            tc.tile_pool(name="f1_ps", bufs=2, space="PSUM"))
                        for g in range(8):
                            w1c = pc.tile([128, ET, 512], F32R,
                                          tag="w1", bufs=2)
                            nc.sync.dma_start(
                                out=w1c,
                                in_=W1_d[:, :, g * 512:(g + 1) * 512])
                            for t in range(4):
                                ft = g * 4 + t
                                f1ps = pr.tile([128, 512], F32,
                                               tag="f1", bufs=4)
                                for et in range(ET):
                                    nc.tensor.matmul(
                                        f1ps,
                                        w1c[:, et, t * 128:(t + 1) * 128],
                                        xlnT[:, et, :],
                                        start=(et == 0), stop=(et == ET - 1))
                                nc.vector.tensor_scalar(
                                    out=f1T[:, ft, :], in0=f1ps,
                                    scalar1=bf1_sb[:, ft:ft + 1], scalar2=0.0,
                                    op0=ALU.add, op1=ALU.max)

                        sC.close()
                        # ==== FFN2 + bf2 + residual + LN3 + out ====
                        pd = sF.enter_context(tc.tile_pool(name="f2_sb", bufs=2))
                        pt2 = sF.enter_context(
                            tc.tile_pool(name="f2_ps", bufs=2, space="PSUM"))
                        f2ps = [pt2.tile([128, 512], F32, tag="f2", bufs=8,
                                         name=f"f2ps{i}")
                                for i in range(8)]
                        for ft in range(FT):
                            w2c = pd.tile([128, E], F32R, tag="w2", bufs=6)
                            nc.sync.dma_start(out=w2c, in_=W2_d[:, ft, :])
                            for qs in range(QT):
                                for eh in range(2):
                                    nc.tensor.matmul(
                                        f2ps[qs * 2 + eh],
                                        f1T[:, ft, qs * 128:(qs + 1) * 128],
                                        w2c[:, eh * 512:(eh + 1) * 512],
                                        start=(ft == 0), stop=(ft == FT - 1))
                        for qs in range(QT):
                            y = pd.tile([128, E], F32, tag="y", bufs=2)
                            for eh in range(2):
                                sl = slice(eh * 512, (eh + 1) * 512)
                                nc.vector.tensor_add(
                                    y[:, sl], f2ps[qs * 2 + eh], bf2_bc[:, sl])
                            nc.vector.tensor_add(y, y, xln[:, qs, :])
                            out_sb = pd.tile([128, E], F32, tag="ysb", bufs=2)
                            layernorm(y, g3_bc, b3_bc, out_sb, pd, "ln3")
                            nc.sync.dma_start(
                                out=out_d[qs * 128:(qs + 1) * 128, :],
                                in_=out_sb)

            if reps == 1:
                body()
            else:
                with tc.For_i(0, reps, 1):
                    body()

    return nc


# ---------------------------------------------------------------------------
# Host side: compile-once runner over PJRT (axon), sharding, gather.
# ---------------------------------------------------------------------------
def _make_runner(nc, n_cores):
    import time

    import jax
    from jax.experimental.shard_map import shard_map
    from jax.sharding import Mesh, PartitionSpec

    from concourse.bass2jax import (
        _bass_exec_p,
        install_neuronx_cc_hook,
        partition_id_tensor,
    )

    install_neuronx_cc_hook()
    partition_name = (
        nc.partition_id_tensor.name if nc.partition_id_tensor else None
    )

    in_names, out_names, out_avals, zero_outs = [], [], [], []
    for alloc in nc.m.functions[0].allocations:
        if not isinstance(alloc, mybir.MemoryLocationSet):
            continue
        name = alloc.memorylocations[0].name
        if alloc.kind == "ExternalInput":
            if name != partition_name:
                in_names.append(name)
        elif alloc.kind == "ExternalOutput":
            shape = tuple(alloc.tensor_shape)
            dtype = mybir.dt.np(alloc.dtype)
            out_names.append(name)
            out_avals.append(jax.core.ShapedArray(shape, dtype))
            zero_outs.append(np.zeros(shape, dtype))
    n_params = len(in_names)
    n_outs = len(out_avals)
    all_in_names = list(in_names) + list(out_names)
    if partition_name is not None:
        all_in_names.append(partition_name)
    donate = tuple(range(n_params, n_params + n_outs))

    def _body(*args):
        operands = list(args)
        if partition_name is not None:
            operands.append(partition_id_tensor())
        outs = _bass_exec_p.bind(
            *operands,
            out_avals=tuple(out_avals),
            in_names=tuple(all_in_names),
            out_names=tuple(out_names),
            lowering_input_output_aliases=(),
            sim_require_finite=True,
            sim_require_nnan=True,
            nc=nc,
        )
        return tuple(outs)

    devices = jax.devices()[:n_cores]
    assert len(devices) == n_cores, f"need {n_cores} cores, saw {len(jax.devices())}"
    mesh = Mesh(np.asarray(devices), ("core",))
    sharded = jax.jit(
        shard_map(
            _body, mesh=mesh,
            in_specs=(PartitionSpec("core"),) * (n_params + n_outs),
            out_specs=(PartitionSpec("core"),) * n_outs,
            check_rep=False,
        ),
        donate_argnums=donate,
        keep_unused=True,
    )

    def run(in_maps, time_reps=0):
        per_core = [[np.asarray(m[name]) for name in in_names]
                    for m in in_maps]
        concat_in = [
            np.concatenate([per_core[c][i] for c in range(n_cores)], axis=0)
            for i in range(n_params)
        ]
        def zeros():
            return [np.zeros((n_cores * z.shape[0], *z.shape[1:]), z.dtype)
                    for z in zero_outs]
        out_arrs = jax.block_until_ready(sharded(*concat_in, *zeros()))
        results = [
            {n: np.asarray(out_arrs[i]).reshape(n_cores,
                                                *out_avals[i].shape)[c]
             for i, n in enumerate(out_names)}
            for c in range(n_cores)
        ]
        times = []
        if time_reps:
            from jax.sharding import NamedSharding
            dev_in = [
                jax.device_put(a, NamedSharding(mesh, PartitionSpec("core")))
                for a in concat_in
            ]
            for _ in range(time_reps):
                z = [jax.device_put(
                        zz, NamedSharding(mesh, PartitionSpec("core")))
                     for zz in zeros()]
                jax.block_until_ready(z)
                t0 = time.perf_counter()
                jax.block_until_ready(sharded(*dev_in, *z))
                times.append(time.perf_counter() - t0)
        return results, times

    return run


_cache = {}


def _get_runner(reps=1):
    key = ("runner", reps)
    if key not in _cache:
        nc = build_nc(reps=reps)
        _cache[key] = _make_runner(nc, N_CORES)
    return _cache[key]


def _shard_inputs(value, query, Wv, Wk, Wq, Wo, bo, g1, b1, W1, bf1, W2,
                  bf2, g3, b3):
    f = lambda a: np.ascontiguousarray(np.asarray(a, np.float32))
    value, query = f(value), f(query)
    Wo_r = np.ascontiguousarray(
        f(Wo).reshape(ET, 128, E).transpose(1, 0, 2))
    W1_r = np.ascontiguousarray(
        f(W1).reshape(ET, 128, FF).transpose(1, 0, 2))
    W2_r = np.ascontiguousarray(
        f(W2).reshape(FT, 128, E).transpose(1, 0, 2))
    shared = {
        "Wq": f(Wq), "Wk": f(Wk), "Wv": f(Wv),
        "Wo_r": Wo_r, "W1_r": W1_r, "W2_r": W2_r,
        "bo": f(bo), "g1": f(g1), "b1": f(b1), "bf1": f(bf1),
        "bf2": f(bf2), "g3": f(g3), "b3": f(b3),
    }
    in_maps = []
    for b in range(B):
        va = np.empty((2, L, H, 65), np.float32)
        for vi in range(2):
            va[vi, :, :, :HD] = value[b, :, vi].reshape(L, H, HD)
            va[vi, :, :, HD] = 1.0
        va_b = va.reshape(2, L, H * 65)
        qb = query[b].reshape(L, E)
        for qs in range(4):
            in_maps.append({
                "v_aug": np.ascontiguousarray(
                    np.roll(va_b, -qs * Q, axis=1)),
                "qres": np.ascontiguousarray(qb[qs * Q:(qs + 1) * Q]),
                **shared,
            })
    return in_maps


def kernel(value, key, query, mask, retrieved_passages,
           Wv, Wk, Wq, Wo, bo, g1, b1, W1, bf1, W2, bf2, g3, b3):
    in_maps = _shard_inputs(value, query, Wv, Wk, Wq, Wo, bo, g1, b1,
                            W1, bf1, W2, bf2, g3, b3)
    run = _get_runner(reps=1)
    results, _ = run(in_maps, 0)
    out = np.empty((B, L, E), np.float32)
    for b in range(B):
        for qs in range(4):
            out[b, qs * Q:(qs + 1) * Q] = results[b * 4 + qs]["out"]
    return out.reshape(B, CN, CL, E)

